# revision 1
# baseline (speedup 1.0000x reference)
"""NT-Xent contrastive loss on 8 Trainium2 NeuronCores.

Math (reference): z = l2-normalize rows of concat(emb_i, emb_j) -> [8192, 512].
sim = (z @ z.T) / T with T = 0.5.  denom_r = sum_j exp(sim_rj) - exp(sim_rr),
sim_rr = 1/T exactly, so subtract e^2.  pos pair sim[k, k+N] = 2*cos_k.
loss = (sum_r log(denom_r) - 4 * sum_k cos_k) / 8192.

Sharding: data-parallel over rows of sim.  Each core computes a 1024-row
block of sim against all 8192 columns, reduces to one partial scalar, plus
a 512-pair slice of the positive-pair cosines.  Host sums the 8 partials.

Device pipeline per core (identical SPMD program, per-core data):
  - stream repsT [512, 8192] f32 (host-transposed) in [128, 2048] tiles
    (8KB DMA bursts per partition line)
  - column sums of squares via ones[128,128]-matmul of squares: the PSUM
    result is REPLICATED across all 128 partitions, so rinv =
    exp(-0.5*ln(ss)) runs at full 128-lane ACT rate straight out of PSUM
    and the Exp output IS the per-column scale tile (no partition
    broadcast, no 1-lane row ops)
  - column scale + bf16 cast in one DVE pass: zT = st_f32 * B -> bf16;
    zT lives in a rotating per-group pool (each 2048-column group is
    consumed by exactly one matmul group)
  - all ACT functions used (Square/Ln/Exp/Copy) are pinned to the single
    natural_log_exp_and_others table set -> one ACT_TABLE_LOAD total
  - main matmul: lhsT = own 1024 normalized cols, rhs = all 8192 cols,
    K=512 over 4 chunks, PSUM groups [128, 2048], bf16
  - ACT exp(2*x) with accum_out -> row sums, ln(denom - e^2), reduce
  - emission is software-pipelined two groups ahead so the strict PE
    FIFO never interleaves a group's prep matmuls behind the mains that
    must overlap them
"""

import functools
import math

import numpy as np

import concourse.bacc as bacc
import concourse.bass as bass
import concourse.tile as tile
from concourse import mybir
from concourse.bass_utils import run_bass_kernel_spmd
from concourse.hw_specs import get_activation_tables as _orig_gat

F32 = mybir.dt.float32
BF16 = mybir.dt.bfloat16
AF = mybir.ActivationFunctionType
ALU = mybir.AluOpType

N_CORES = 8
N = 4096              # rows per input
D = 512               # embedding dim
M = 2 * N             # 8192 rows of sim
ROWS_PER_CORE = M // N_CORES      # 1024
POS_PER_CORE = N // N_CORES       # 512
D_CH = D // 128       # 4 contraction chunks
E2 = float(math.exp(2.0))
INV_T = 2.0           # 1 / temperature
GW = 2048             # column-group width

_ONE_SET = "natural_log_exp_and_others"


@functools.cache
def _patched_gat(arch):
    """Pin every ACT function this kernel uses to one table set so the
    table-load chooser emits a single ACT_TABLE_LOAD (the default
    first-match policy alternates sets on every Ln<->Exp transition,
    costing ~2.7us per switch)."""
    t = dict(_orig_gat(arch))
    if _ONE_SET not in t:
        return t
    mine = {AF.Exp, AF.Ln, AF.Square, AF.Copy, AF.Identity}
    return {
        name: (s if name == _ONE_SET else (set(s) - mine))
        for name, s in t.items()
    }


def build_program():
    bacc.get_activation_tables = _patched_gat

    nc = bacc.Bacc(
        "TRN2",
        target_bir_lowering=False,
        debug=False,
        num_devices=N_CORES,
    )

    repsT = nc.dram_tensor("repsT", [D, M], F32, kind="ExternalInput")
    myT = nc.dram_tensor("myT", [D, ROWS_PER_CORE], F32, kind="ExternalInput")
    pi = nc.dram_tensor("pi", [POS_PER_CORE, D], F32, kind="ExternalInput")
    pj = nc.dram_tensor("pj", [POS_PER_CORE, D], F32, kind="ExternalInput")
    out_d = nc.dram_tensor("out", [2, 1], F32, kind="ExternalOutput")

    with tile.TileContext(nc) as tc:
        import contextlib

        with contextlib.ExitStack() as ctx:
            const = ctx.enter_context(tc.tile_pool(name="const", bufs=1))
            big = ctx.enter_context(tc.tile_pool(name="big", bufs=1))
            stage = ctx.enter_context(tc.tile_pool(name="stage", bufs=9))
            sqp = ctx.enter_context(tc.tile_pool(name="sqp", bufs=3))
            bpool = ctx.enter_context(tc.tile_pool(name="bpool", bufs=3))
            lnp = ctx.enter_context(tc.tile_pool(name="lnp", bufs=6))
            ztp = ctx.enter_context(tc.tile_pool(name="ztp", bufs=2))
            posp = ctx.enter_context(tc.tile_pool(name="posp", bufs=4))
            sink = ctx.enter_context(tc.tile_pool(name="sink", bufs=2))
            esink = ctx.enter_context(tc.tile_pool(name="esink", bufs=2))

            ones128 = const.tile([128, 128], BF16)
            nc.vector.memset(ones128[:], 1.0)
            ones_f = const.tile([128, 1], F32)
            nc.vector.memset(ones_f[:], 1.0)
            neg_e2 = const.tile([128, 1], F32)
            nc.vector.memset(neg_e2[:], -E2)

            lhsT = [big.tile([128, ROWS_PER_CORE], BF16, tag=f"lhsT{d}",
                             name=f"lhsT{d}") for d in range(D_CH)]
            dacc = big.tile([128, 32], F32, tag="dacc")
            pos_ssi = big.tile([128, 4], F32, tag="pos_ssi")
            pos_ssj = big.tile([128, 4], F32, tag="pos_ssj")
            pos_dot = big.tile([128, 4], F32, tag="pos_dot")

            pp_main = ctx.enter_context(
                tc.tile_pool(name="pp_main", bufs=2, space="PSUM")
            )

            def emit_prep_group(src, col0, w, dst, label):
                """Normalize w columns of src starting at col0 into dst
                (4 chunk tiles [128, w] bf16).  w in {1024, 2048}."""
                nk = w // 512
                pt = pp_main.tile([128, GW], F32, tag="pp_main",
                                  name=f"ssg_{label}")
                sts = []
                for d in range(D_CH):
                    st = stage.tile([128, GW], F32, tag="stage",
                                    name=f"st_{label}_{d}")
                    nc.sync.dma_start(
                        st[0:128, 0:w], src[bass.ts(d, 128), col0 : col0 + w]
                    )
                    sts.append(st)
                    sqt = sqp.tile([128, GW], BF16, tag="sqp",
                                   name=f"sq_{label}_{d}")
                    if d < 2:
                        nc.scalar.activation(sqt[0:128, 0:w], st[0:128, 0:w],
                                             AF.Square)
                    else:
                        nc.vector.tensor_mul(sqt[0:128, 0:w], st[0:128, 0:w],
                                             st[0:128, 0:w])
                    for k in range(nk):
                        nc.tensor.matmul(
                            pt[:, bass.ts(k, 512)],
                            ones128[:], sqt[:, bass.ts(k, 512)],
                            start=(d == 0), stop=(d == D_CH - 1),
                        )
                bt = bpool.tile([128, GW], BF16, tag="bpool",
                                name=f"B_{label}")
                for k in range(nk):
                    lt = lnp.tile([128, 512], F32, tag="lnp")
                    nc.scalar.activation(lt[:], pt[:, bass.ts(k, 512)], AF.Ln)
                    nc.scalar.activation(bt[:, bass.ts(k, 512)], lt[:],
                                         AF.Exp, scale=-0.5)
                for d in range(D_CH):
                    nc.vector.tensor_mul(
                        dst[d][0:128, 0:w], sts[d][0:128, 0:w],
                        bt[0:128, 0:w],
                    )

            def new_zgroup(jg):
                return [ztp.tile([128, GW], BF16, tag=f"zt{d}",
                                 name=f"zt_{jg}_{d}") for d in range(D_CH)]

            def emit_mains(jg, zg):
                for i in range(8):
                    pt = pp_main.tile([128, GW], F32, tag="pp_main",
                                      name=f"mm_{jg}_{i}")
                    for d in range(D_CH):
                        for jj in range(4):
                            nc.tensor.matmul(
                                pt[:, bass.ts(jj, 512)],
                                lhsT[d][:, bass.ts(i, 128)],
                                zg[d][:, bass.ts(jj, 512)],
                                start=(d == 0), stop=(d == D_CH - 1),
                            )
                    es = esink.tile([128, GW], BF16, tag="esink")
                    k = i * 4 + jg
                    nc.scalar.activation(
                        es[:], pt[:], AF.Exp, scale=INV_T,
                        accum_out=dacc[:, k : k + 1],
                    )

            def emit_pos():
                for t in range(4):
                    pit = posp.tile([128, D], F32, tag="posp")
                    nc.sync.dma_start(pit[:], pi[bass.ts(t, 128), :])
                    pjt = posp.tile([128, D], F32, tag="posp")
                    nc.sync.dma_start(pjt[:], pj[bass.ts(t, 128), :])
                    for src0, src1, acc in (
                        (pit, pit, pos_ssi),
                        (pjt, pjt, pos_ssj),
                        (pit, pjt, pos_dot),
                    ):
                        snk = sink.tile([128, D], F32, tag="sink")
                        nc.vector.tensor_mul(snk[:], src0[:], src1[:])
                        nc.vector.tensor_reduce(
                            acc[:, t : t + 1], snk[:],
                            axis=mybir.AxisListType.X, op=ALU.add,
                        )
                lssi = big.tile([128, 4], F32, tag="lssi")
                lssj = big.tile([128, 4], F32, tag="lssj")
                nc.scalar.activation(lssi[:], pos_ssi[:], AF.Ln)
                nc.scalar.activation(lssj[:], pos_ssj[:], AF.Ln)
                lsum = big.tile([128, 4], F32, tag="lsum")
                nc.vector.tensor_add(lsum[:], lssi[:], lssj[:])
                rinv_ij = big.tile([128, 4], F32, tag="rinv_ij")
                nc.scalar.activation(rinv_ij[:], lsum[:], AF.Exp, scale=-0.5)
                posk = big.tile([128, 4], F32, tag="posk")
                nc.vector.tensor_mul(posk[:], pos_dot[:], rinv_ij[:])
                return posk

            # ------- software-pipelined schedule ----------------------------
            emit_prep_group(myT, 0, ROWS_PER_CORE, lhsT, "my")
            zg = {}
            zg[0] = new_zgroup(0)
            emit_prep_group(repsT, 0, GW, zg[0], "g0")
            zg[1] = new_zgroup(1)
            emit_prep_group(repsT, GW, GW, zg[1], "g1")
            emit_mains(0, zg[0])
            zg[2] = new_zgroup(2)
            emit_prep_group(repsT, 2 * GW, GW, zg[2], "g2")
            posk = emit_pos()
            emit_mains(1, zg[1])
            zg[3] = new_zgroup(3)
            emit_prep_group(repsT, 3 * GW, GW, zg[3], "g3")
            emit_mains(2, zg[2])
            emit_mains(3, zg[3])

            # ------- final reduction ----------------------------------------
            dn = big.tile([128, 8], F32, tag="dn")
            nc.vector.tensor_reduce(
                dn[:], dacc[:].rearrange("p (i g) -> p i g", g=4),
                axis=mybir.AxisListType.X, op=ALU.add,
            )
            ld = big.tile([128, 8], F32, tag="ld")
            nc.scalar.activation(ld[:], dn[:], AF.Ln, bias=neg_e2[:])
            fin = big.tile([128, 2], F32, tag="fin")
            nc.vector.tensor_reduce(
                fin[:, 0:1], ld[:], axis=mybir.AxisListType.X, op=ALU.add
            )
            nc.vector.tensor_reduce(
                fin[:, 1:2], posk[:], axis=mybir.AxisListType.X, op=ALU.add
            )
            fmm = pp_main.tile([128, GW], F32, tag="pp_main", name="fmm")
            nc.tensor.matmul(fmm[0:2, 0:1], fin[:], ones_f[:], start=True,
                             stop=True)
            outsb = big.tile([2, 1], F32, tag="outsb")
            nc.vector.tensor_copy(outsb[:], fmm[0:2, 0:1])
            nc.sync.dma_start(out_d[:], outsb[:])

    nc.compile()
    return nc


_NC_CACHE = None


def _get_program():
    global _NC_CACHE
    if _NC_CACHE is None:
        _NC_CACHE = build_program()
    return _NC_CACHE


def make_in_maps(emb_i: np.ndarray, emb_j: np.ndarray):
    emb_i = np.asarray(emb_i, dtype=np.float32)
    emb_j = np.asarray(emb_j, dtype=np.float32)
    reps = np.concatenate([emb_i, emb_j], axis=0)          # [8192, 512]
    repsT = np.ascontiguousarray(reps.T)                   # [512, 8192]
    in_maps = []
    for c in range(N_CORES):
        in_maps.append(
            {
                "repsT": repsT,
                "myT": np.ascontiguousarray(
                    repsT[:, c * ROWS_PER_CORE : (c + 1) * ROWS_PER_CORE]
                ),
                "pi": np.ascontiguousarray(
                    emb_i[c * POS_PER_CORE : (c + 1) * POS_PER_CORE]
                ),
                "pj": np.ascontiguousarray(
                    emb_j[c * POS_PER_CORE : (c + 1) * POS_PER_CORE]
                ),
            }
        )
    return in_maps


def combine_outputs(results):
    ld_sum = 0.0
    cos_sum = 0.0
    for r in results:
        o = np.asarray(r["out"], dtype=np.float64).reshape(-1)
        ld_sum += o[0]
        cos_sum += o[1]
    loss = (ld_sum - 2.0 * INV_T * cos_sum) / float(M)
    return np.float32(loss)


def kernel(emb_i: np.ndarray, emb_j: np.ndarray) -> np.ndarray:
    nc = _get_program()
    in_maps = make_in_maps(emb_i, emb_j)
    res = run_bass_kernel_spmd(nc, in_maps, list(range(N_CORES)))
    return combine_outputs(res.results)



# revision 2
# speedup vs baseline: 3.0940x; 3.0940x over previous
"""NT-Xent contrastive loss on 8 Trainium2 NeuronCores — symmetric fp8 version.

Math: z = l2-normalize rows of concat(emb_i, emb_j) -> [8192, 512].
sim = (z @ z.T)/T, T=0.5.  denom_r = sum_j exp(sim_rj) - exp(sim_rr).
loss = (sum_r log denom_r - 4*sum_k cos_k) / 8192.

exp(sim) is symmetric, so only the upper triangle of the 16x16 grid of
512-row strip pairs (136 pairs) is computed, split 17 pairs/core:
core c owns row strips A=2c, B=2c+1 and computes blocks against 10
column strips (A, B, the next 6 strips cyclically, and 2 "far" strips
x,y that split the distance-4 superblock between core pairs).  Row
sums of exp blocks come free via ACT accum_out; the mirror (column)
sums are accumulated elementwise into bf16 SBUF tiles by DVE and
reduced over the partition axis on the host (8 cores x 576KB, cheap).

Host does the O(N*D) work exactly in f64: normalization, fp8(e4m3)
quantization (x64 scale, matching TRN FP8_EXP4 encodings for |v|<240),
positive-pair cosines, the exact per-row self-term exp(2*||q_r||^2),
final log/assembly.  Device does only the O(N^2 D) matmul + exp:
DoubleRow fp8 matmuls (K=512 as 2 double-chunks of the [128,4,cols]
ksub layout), 2x PE throughput vs bf16.
"""

import numpy as np
import ml_dtypes

import concourse.bacc as bacc
import concourse.bass as bass
import concourse.tile as tile
from concourse import mybir
from concourse.bass_utils import run_bass_kernel_spmd

F32 = mybir.dt.float32
BF16 = mybir.dt.bfloat16
F8 = mybir.dt.float8e4
AF = mybir.ActivationFunctionType
ALU = mybir.AluOpType
ts = bass.ts

N_CORES = 8
N = 4096
D = 512
M = 2 * N
SW = 512                 # strip width (rows)
KSUB = D // 128          # 4 k-subtiles of 128
SCALE = 64.0             # fp8 quantization scale for z
ACT_SCALE = 2.0 / (SCALE * SCALE)   # exp(sim_psum * ACT_SCALE) = exp(2*cos)
NCOL = 10 * SW           # local columns per core

# local col-tile order in zt / SBUF:  T0 T1 T2 T3 T9 | T4 T5 T6 T7 | T8
TILE_OFF = {0: 0, 1: 512, 2: 1024, 3: 1536, 9: 2048,
            4: 2560, 5: 3072, 6: 3584, 7: 4096, 8: 4608}
CH_A = 2560   # first DMA chunk: T0..T3, T9
CH_B = 2048   # second: T4..T7
CH_C = 512    # third: T8


def strips_for_core(c):
    base = [(2 * c + i) % 16 for i in range(8)]
    if c < 4:
        x, y = 2 * c + 8, 2 * c + 9
    else:
        x, y = 2 * c - 7, 2 * c - 8
    return base + [x, y]


def build_program():
    nc = bacc.Bacc(
        "TRN2",
        target_bir_lowering=False,
        debug=False,
        num_devices=N_CORES,
    )

    zt_d = nc.dram_tensor("zt", [128, KSUB, NCOL], F8, kind="ExternalInput")
    rs_d = nc.dram_tensor("rs", [128, 8], F32, kind="ExternalOutput")
    acc0_d = nc.dram_tensor("acc0", [128, 2048], BF16, kind="ExternalOutput")
    acc1_d = nc.dram_tensor("acc1", [128, 2048], BF16, kind="ExternalOutput")
    acc2_d = nc.dram_tensor("acc2", [128, 512], BF16, kind="ExternalOutput")

    DR = mybir.MatmulPerfMode.DoubleRow

    with tile.TileContext(nc) as tc:
        import contextlib

        with contextlib.ExitStack() as ctx:
            big = ctx.enter_context(tc.tile_pool(name="big", bufs=1))
            esp = ctx.enter_context(tc.tile_pool(name="esp", bufs=3))
            pp = ctx.enter_context(
                tc.tile_pool(name="pp", bufs=2, space="PSUM")
            )

            ztA = big.tile([128, KSUB, CH_A], F8, tag="ztA")
            ztB = big.tile([128, KSUB, CH_B], F8, tag="ztB")
            ztC = big.tile([128, KSUB, CH_C], F8, tag="ztC")
            dacc = big.tile([128, 32], F32, tag="dacc")
            acc0 = big.tile([128, 2048], BF16, tag="acc0")
            acc1 = big.tile([128, 2048], BF16, tag="acc1")
            acc2 = big.tile([128, 512], BF16, tag="acc2")
            rs = big.tile([128, 8], F32, tag="rs")

            nc.vector.memset(dacc[:], 0.0)
            nc.vector.memset(acc0[:], 0.0)
            nc.vector.memset(acc1[:], 0.0)
            nc.vector.memset(acc2[:], 0.0)

            nc.sync.dma_start(ztA[:], zt_d[:, :, 0:CH_A])
            nc.sync.dma_start(ztB[:], zt_d[:, :, CH_A : CH_A + CH_B])
            nc.sync.dma_start(ztC[:], zt_d[:, :, CH_A + CH_B : NCOL])

            def chunk_of(t):
                off = TILE_OFF[t]
                if off < CH_A:
                    return ztA, off
                if off < CH_A + CH_B:
                    return ztB, off - CH_A
                return ztC, off - CH_A - CH_B

            def emit_wave(wname, row, wslot, tiles_):
                """One wave: 4 rowgroups of 128 rows from strip `row`
                (0=A,1=B) x the col tiles in tiles_ (psum slot i = tiles_[i]).
                wslot = dacc wave slot. Emits MMs + exp/accum + DVE adds."""
                nt = len(tiles_)
                w = nt * 512
                for g in range(4):
                    lhs_off = row * 512 + g * 128
                    pt = pp.tile([128, 2048], F32, tag="pp",
                                 name=f"pt_{wname}_{g}")
                    for k in range(2):
                        lhsT = ztA[:, 2 * k : 2 * k + 2,
                                   lhs_off : lhs_off + 128]
                        for i, t in enumerate(tiles_):
                            cht, choff = chunk_of(t)
                            rhs = cht[:, 2 * k : 2 * k + 2,
                                      choff : choff + 512]
                            nc.tensor.matmul(
                                pt[:, ts(i, 512)], lhsT, rhs,
                                start=(k == 0), stop=(k == 1),
                                perf_mode=DR,
                            )
                    es = esp.tile([128, 2048], BF16, tag="esp",
                                  name=f"es_{wname}_{g}")
                    gslot = (row * 4 + g) * 4 + wslot
                    nc.scalar.activation(
                        es[0:128, 0:w], pt[0:128, 0:w], AF.Exp,
                        scale=ACT_SCALE,
                        accum_out=dacc[:, gslot : gslot + 1],
                    )
                    yield g, es

            # ---- A-W0: rows A x [T0 T1 T2 T3]; mirrors T1..T3 ----
            for g, es in emit_wave("aw0", 0, 0, [0, 1, 2, 3]):
                nc.vector.tensor_add(
                    acc0[:, 0:1536], acc0[:, 0:1536], es[:, 512:2048]
                )
            # ---- B-W0: rows B x [T1 T2 T3 T9]; mirrors T2 T3 T9 ----
            for g, es in emit_wave("bw0", 1, 0, [1, 2, 3, 9]):
                nc.vector.tensor_add(
                    acc0[:, 512:2048], acc0[:, 512:2048], es[:, 512:2048]
                )
            nc.sync.dma_start(acc0_d[:], acc0[:])

            # ---- A-W1 / B-W1: rows x [T4 T5 T6 T7]; all mirrors ----
            for g, es in emit_wave("aw1", 0, 1, [4, 5, 6, 7]):
                nc.vector.tensor_add(acc1[:], acc1[:], es[:, 0:2048])
            for g, es in emit_wave("bw1", 1, 1, [4, 5, 6, 7]):
                nc.vector.tensor_add(acc1[:], acc1[:], es[:, 0:2048])
            nc.sync.dma_start(acc1_d[:], acc1[:])

            # ---- A-W2: rows A x [T8]; mirror T8 ----
            for g, es in emit_wave("aw2", 0, 2, [8]):
                nc.vector.tensor_add(acc2[:], acc2[:], es[:, 0:512])
            nc.sync.dma_start(acc2_d[:], acc2[:])

            # ---- rowsum finale ----
            nc.vector.tensor_reduce(
                rs[:], dacc[:].rearrange("p (g w) -> p g w", w=4),
                axis=mybir.AxisListType.X, op=ALU.add,
            )
            nc.sync.dma_start(rs_d[:], rs[:])

    nc.compile()
    return nc


_NC_CACHE = None


def _get_program():
    global _NC_CACHE
    if _NC_CACHE is None:
        _NC_CACHE = build_program()
    return _NC_CACHE


def quantize_z(emb_i: np.ndarray, emb_j: np.ndarray):
    """Host-side exact prep: returns (q8 [8192,512] fp8, pos_sum, selfterm)."""
    reps = np.concatenate(
        [np.asarray(emb_i, np.float64), np.asarray(emb_j, np.float64)], 0
    )
    z = reps / np.linalg.norm(reps, axis=1, keepdims=True)
    q8 = (z * SCALE).astype(np.float32).astype(ml_dtypes.float8_e4m3)
    qf = q8.astype(np.float64) / SCALE
    pos_sum = float((z[:N] * z[N:]).sum())
    selfterm = np.exp(2.0 * (qf * qf).sum(1))        # device's own diag entry
    return q8, pos_sum, selfterm


def make_in_maps(q8: np.ndarray):
    # zt[p, ksub, col] = q8[global_col_row, ksub*128 + p]
    qT = np.ascontiguousarray(q8.T).reshape(KSUB, 128, M)  # [ksub, p, row]
    in_maps = []
    for c in range(N_CORES):
        S = strips_for_core(c)
        order = [S[0], S[1], S[2], S[3], S[9], S[4], S[5], S[6], S[7], S[8]]
        cols = np.concatenate(
            [np.arange(s * SW, (s + 1) * SW) for s in order]
        )
        zt = np.ascontiguousarray(
            qT[:, :, cols].transpose(1, 0, 2)
        )  # [128, KSUB, NCOL]
        in_maps.append({"zt": zt})
    return in_maps


def combine_outputs(results, pos_sum, selfterm):
    denom = np.zeros(M, np.float64)
    for c in range(N_CORES):
        S = strips_for_core(c)
        A, B = S[0], S[1]
        rs = np.asarray(results[c]["rs"], np.float64)        # [128, 8]
        denom[A * SW : (A + 1) * SW] += rs[:, 0:4].T.reshape(SW)
        denom[B * SW : (B + 1) * SW] += rs[:, 4:8].T.reshape(SW)
        cs0 = np.asarray(results[c]["acc0"], np.float64).sum(0)   # [2048]
        cs1 = np.asarray(results[c]["acc1"], np.float64).sum(0)
        cs2 = np.asarray(results[c]["acc2"], np.float64).sum(0)
        for i, t in enumerate([1, 2, 3, 9]):
            g = S[t]
            denom[g * SW : (g + 1) * SW] += cs0[i * 512 : (i + 1) * 512]
        for i, t in enumerate([4, 5, 6, 7]):
            g = S[t]
            denom[g * SW : (g + 1) * SW] += cs1[i * 512 : (i + 1) * 512]
        g = S[8]
        denom[g * SW : (g + 1) * SW] += cs2[0:512]
    denom -= selfterm
    loss = (np.log(denom).sum() - 4.0 * pos_sum) / float(M)
    return np.float32(loss)


def kernel(emb_i: np.ndarray, emb_j: np.ndarray) -> np.ndarray:
    nc = _get_program()
    q8, pos_sum, selfterm = quantize_z(emb_i, emb_j)
    in_maps = make_in_maps(q8)
    res = run_bass_kernel_spmd(nc, in_maps, list(range(N_CORES)))
    return combine_outputs(res.results, pos_sum, selfterm)
